# revision 11
# baseline (speedup 1.0000x reference)
"""Trainium2 Bass kernel for RoPE'd causal attention (no softmax).

Reference computation (B=2, H=8, T=2048, N=512, DV=128):
    QR = Q*cos + rotate_half_interleaved(Q)*sin         (K == Q)
    S  = QR @ QR^T          [B,H,T,T]
    S  = tril(S, -1)        (strictly lower triangular)
    O  = S @ V              [B,H,T,DV]

Sharding: the 16 (b,h) pairs are split 2-per-core across 8 NeuronCores.

v2 design (vs the fp32r baseline at ~144 us):
  - All matmul operands are fp16: Q and V are converted to fp16 on host
    (the 2e-2 rel-err budget dwarfs fp16's ~2.4e-4 quantization), which
    halves Q DMA traffic, makes every RoPE vector op a 2-byte op (DVE
    2x_1p mode, 2x throughput), runs matmuls at 1 cyc/row at ANY moving
    width (fp32r pays 4 cyc/row below 256), and transposes at 1.0
    cyc/row (vs 1.5).
  - 256-wide column groups (8 per bh) instead of 512: score work
    becomes available to the PE earlier in the startup ramp, which is
    what kept the HAM clock gate at 1.2 GHz for the first ~50 us of the
    baseline.
  - AV matmuls use the masked score block S^T[s, t-chunk] as the
    STATIONARY operand and V[s,:] as moving, accumulating O[t, d]
    directly in PSUM -- no output transposes and no [d,t]->[t,d] copies.
  - Phase A (DMA+RoPE+transpose) of group g+1 is emitted BEFORE phase
    B/C of group g so the engine FIFOs never serialize the PE behind
    the RoPE of the next group.
  - DMA enqueues are spread across the sync (q, out), scalar (first
    tables), gpsimd (later tables) and vector (V) rings; q tiles are
    prefetched two groups ahead.
  - Optional PE warm-up matmuls (KERNEL_WARM=n) right after the
    constants, to pull the HAM 2.4 GHz grant earlier.
"""

import math
import os

import numpy as np

B, H, T, NDIM, DV = 2, 8, 2048, 512, 128
P = 128            # partitions
NT = T // P        # 16 t-tiles per (b,h)
NG = 8             # t-groups per (b,h)
GW = T // NG       # 256 group width
TPG = GW // P      # 2 tiles per group
NK = NDIM // P     # 4 contraction chunks
NCORES = 8
BH_PER_CORE = (B * H) // NCORES  # 2

WARM = int(os.environ.get("KERNEL_WARM", "0"))

TRACE = False          # set by test harness to capture HW profile
LAST_RESULTS = None    # BassKernelResults of the last kernel() call

_NC_CACHE = {}


def _host_tables(freqs):
    """Mirror reference.py's fp32 phase arithmetic exactly."""
    f = np.asarray(freqs, dtype=np.float32).reshape(NDIM)
    t = np.arange(T, dtype=np.float32)
    ph = t[:, None] * f[None, :]            # fp32 multiply, like jnp
    ph = ph % np.float32(1.0)
    ph = ph * np.float32(2.0 * math.pi)
    cosv = np.cos(ph).astype(np.float32)
    sinv = np.sin(ph).astype(np.float32)
    # tmp = Q_pairswapped * ssw gives rotate_half(Q) * sin:
    #   ssw[t, 2i]   = -sin[t, 2i]
    #   ssw[t, 2i+1] = +sin[t, 2i+1]
    sign = np.tile(np.array([-1.0, 1.0], dtype=np.float32), NDIM // 2)
    ssw = sinv * sign[None, :]
    return cosv.astype(np.float16), np.ascontiguousarray(ssw).astype(np.float16)


def _emit(tc, nc, aps):
    import concourse.mybir as mybir
    from contextlib import ExitStack
    from concourse.bass import ds, ts

    q, v, cosd, sswd, o = aps
    f32 = mybir.dt.float32
    f16 = mybir.dt.float16

    with ExitStack() as ctx:

        def pool(name, bufs, space="SBUF"):
            return ctx.enter_context(
                tc.tile_pool(name=name, bufs=bufs, space=space)
            )

        const = pool("const", 1)
        cospool = pool("cost", NT)
        sswpool = pool("sswt", NT)
        qin = pool("qin", 12)
        qrp = pool("qr", 3)
        tmpp = pool("tmp", 3)
        qrtp = pool("qrt", BH_PER_CORE * NG)
        stp = pool("st", 6)
        vp = pool("v", 2)
        outp = pool("out", 4)
        # PSUM pool buffers are bank-granular (one 2 KB bank each, 8 total).
        # A PSUM bank holds a single accumulation group: any start=True into
        # a bank re-opens its group, turning the next write of an open chain
        # into an overwrite (measured on HW) -- hence exactly one start per
        # pso tile, below.
        ps_tr = pool("pstr", 2, "PSUM")
        ps_s = pool("pss", 3, "PSUM")
        ps_o = pool("pso", 3, "PSUM")

        # Constants are built on the otherwise-idle GpSimd engine.
        ident = const.tile([P, P], f16, name="ident")
        nc.gpsimd.memset(ident[:], 0.0)
        nc.gpsimd.affine_select(
            out=ident[:],
            in_=ident[:],
            compare_op=mybir.AluOpType.not_equal,
            fill=1.0,
            base=0,
            pattern=[[-1, P]],
            channel_multiplier=1,
        )
        # mask[sp, x] = 1.0 iff sp < x; diagonal block d uses mask[:, :GW-128d]
        # against pss[:, 128d:].
        mask = const.tile([P, GW], f16, name="mask")
        nc.gpsimd.memset(mask[:], 1.0)
        nc.gpsimd.affine_select(
            out=mask[:],
            in_=mask[:],
            compare_op=mybir.AluOpType.is_ge,
            fill=0.0,
            base=-1,
            pattern=[[1, GW]],
            channel_multiplier=-1,
        )

        if WARM:
            # Real (non-transpose) matmuls count toward the HAM busy clock;
            # bridge the DMA/RoPE-bound startup so the 2.4 GHz grant lands
            # by the time the first score blocks issue.
            wps = ps_s.tile([P, GW], f32, name="warm")
            for _ in range(WARM):
                nc.tensor.matmul(
                    wps[:], ident[:], mask[:],
                    start=True, stop=True, skip_group_check=True,
                )

        cosr = cosd.rearrange("(j p) n -> j p n", p=P)
        sswr = sswd.rearrange("(j p) n -> j p n", p=P)
        qr_ = q.rearrange("b (j p) n -> b j p n", p=P)    # [2,16,128,512]
        vr = v.rearrange("b (i s) d -> b s i d", s=P)     # [2,128,16,128]

        cos_t = [None] * NT
        ssw_t = [None] * NT

        def load_tables(j, eng):
            ct = cospool.tile([P, NDIM], f16)
            eng.dma_start(ct[:], cosr[j])
            st_ = sswpool.tile([P, NDIM], f16)
            eng.dma_start(st_[:], sswr[j])
            cos_t[j] = ct
            ssw_t[j] = st_

        # Tables are prefetched just-in-time (2 groups ahead, like q):
        # enqueuing all 4 MB of tables upfront starves the first groups'
        # critical DMAs of bandwidth and stalls the PE for ~15 us.
        def prefetch_tables(g):
            eng = nc.scalar if g < 2 else nc.gpsimd
            for jj in range(TPG):
                load_tables(TPG * g + jj, eng)

        q_t = {}

        def prefetch_q(g):
            for bh in range(BH_PER_CORE):
                for jj in range(TPG):
                    j = TPG * g + jj
                    qt = qin.tile([P, NDIM], f16)
                    nc.sync.dma_start(qt[:], qr_[bh, j])
                    q_t[(bh, j)] = qt

        prefetch_q(0)
        prefetch_tables(0)
        prefetch_q(1)
        prefetch_tables(1)

        # V rides the gpsimd ring (ahead of the later tables): needed from
        # BC(0)'s AV on.
        v_sbs = []
        for b_ in range(BH_PER_CORE):
            vt = vp.tile([P, NT, DV], f16, name=f"v_sb{b_}")
            nc.gpsimd.dma_start(vt[:], vr[b_])
            v_sbs.append(vt)

        qrt = [[None] * NG for _ in range(BH_PER_CORE)]

        def emit_a(g):
            """DMA+RoPE+transpose the TPG tiles of group g for both bh."""
            if g + 2 < NG:
                prefetch_q(g + 2)
                prefetch_tables(g + 2)
            for bh in range(BH_PER_CORE):
                qrt_g = qrtp.tile([P, NK, GW], f16)
                qrt[bh][g] = qrt_g
                for jj in range(TPG):
                    j = TPG * g + jj
                    qt = q_t.pop((bh, j))
                    qr_tile = qrp.tile([P, NDIM], f16)
                    tmp = tmpp.tile([P, NDIM], f16)
                    nc.vector.tensor_mul(qr_tile[:], qt[:], cos_t[j][:])
                    qsw = qt.rearrange("p (a two) -> p a two", two=2)[:, :, ::-1]
                    nc.vector.tensor_tensor(
                        tmp.rearrange("p (a two) -> p a two", two=2),
                        qsw,
                        ssw_t[j].rearrange("p (a two) -> p a two", two=2),
                        mybir.AluOpType.mult,
                    )
                    nc.vector.tensor_add(qr_tile[:], qr_tile[:], tmp[:])
                    pst = ps_tr.tile([P, NK, P], f16)
                    for nk in range(NK):
                        nc.tensor.transpose(
                            pst[:, nk, :], qr_tile[:, ts(nk, P)], ident[:]
                        )
                    nc.vector.tensor_copy(qrt_g[:, :, ts(jj, P)], pst[:])

        pending_av = None   # previous group's final AV matmuls, deferred
        pending_out = None  # previous group's output copy+DMA, deferred

        def emit_bc(bh, g):
            """Scores + AV accumulation for group g of one bh.

            Block (i, g) computes S^T[s in tile i, t in group g].  Diagonal
            straddlers (d = i - 2g >= 0) only live in columns [128d:] and
            get the strict-lower mask; their AV skips all-zero t-chunks.
            The AV matmul uses the masked score chunk as the stationary
            operand so O[t, d] accumulates directly -- no output transpose.
            """
            nonlocal pending_av, pending_out
            v_sb = v_sbs[bh]
            qrt_g = qrt[bh][g]
            pso = ps_o.tile([P, TPG, DV], f32)
            ns = TPG * g + TPG  # number of s-tiles for this group
            av_args = []
            first_av = [True]

            def emit_av(i):
                # Exactly ONE start=True per pso bank: it opens the bank's
                # accumulation group; the first write to each address within
                # the open group overwrites (never reads stale PSUM), later
                # writes accumulate.  A second start would re-open the group
                # and turn the next accumulate into an overwrite.
                st_i, lo_i = av_args[i]
                for c in range(TPG):
                    if P * c < lo_i:  # all-zero chunk of a d=1 block
                        continue
                    nc.tensor.matmul(
                        pso[:, c, :],
                        st_i[:, ts(c, P)],
                        v_sb[:, i, :],
                        start=first_av[0],
                        stop=(i == ns - 1 and c == TPG - 1),
                        skip_group_check=True,
                    )
                    first_av[0] = False

            for i in range(ns):
                d = i - TPG * g
                lo = P * d if d > 0 else 0
                pss = ps_s.tile([P, GW], f32)
                gi, ii = i // TPG, i % TPG
                for nk in range(NK):
                    nc.tensor.matmul(
                        pss[:, lo:],
                        qrt[bh][gi][:, nk, ts(ii, P)],
                        qrt_g[:, nk, lo:],
                        start=(nk == 0),
                        stop=(nk == NK - 1),
                        skip_group_check=True,
                    )
                st_t = stp.tile([P, GW], f16)
                if d >= 0:  # diagonal-straddling block: strict-lower mask
                    nc.vector.tensor_tensor(
                        st_t[:, lo:],
                        pss[:, lo:],
                        mask[:, : GW - lo],
                        mybir.AluOpType.mult,
                    )
                else:
                    nc.scalar.copy(st_t[:], pss[:])
                av_args.append((st_t, lo))
                if i == 0 and pending_av is not None:
                    pending_av()
                    pending_av = None
                if i == 1 and pending_out is not None:
                    pending_out()
                    pending_out = None
                if i > 0:  # AV lags one block so the mask/copy can finish
                    emit_av(i - 1)
            pending_av = lambda n_=ns - 1, f_=emit_av: f_(n_)  # noqa: E731

            def out_thunk(bh_=bh, g_=g, pso_=pso):
                out_sb = outp.tile([P, TPG, DV], f32)
                nc.scalar.copy(out_sb[:], pso_[:])
                dst = o[bh_, ds(g_ * GW, GW), :].rearrange(
                    "(c tp) d -> tp c d", tp=P
                )
                nc.sync.dma_start(dst, out_sb[:])

            pending_out = out_thunk

        emit_a(0)
        emit_a(1)
        for g in range(NG):
            for bh in range(BH_PER_CORE):
                emit_bc(bh, g)
            if g + 2 < NG:
                emit_a(g + 2)
        pending_av()
        pending_out()


def build_nc():
    import concourse.bass as bass  # noqa: F401
    import concourse.mybir as mybir
    import concourse.tile as tile
    from concourse import bacc

    nc = bacc.Bacc(
        "TRN2",
        target_bir_lowering=False,
        debug=False,
        enable_asserts=False,
        num_devices=NCORES,
    )
    f32 = mybir.dt.float32
    f16 = mybir.dt.float16
    q = nc.dram_tensor("q", [BH_PER_CORE, T, NDIM], f16, kind="ExternalInput").ap()
    v = nc.dram_tensor("v", [BH_PER_CORE, T, DV], f16, kind="ExternalInput").ap()
    cosd = nc.dram_tensor("cosv", [T, NDIM], f16, kind="ExternalInput").ap()
    sswd = nc.dram_tensor("ssw", [T, NDIM], f16, kind="ExternalInput").ap()
    o = nc.dram_tensor("o", [BH_PER_CORE, T, DV], f32, kind="ExternalOutput").ap()

    with tile.TileContext(nc) as tc:
        _emit(tc, nc, (q, v, cosd, sswd, o))
    nc.compile()
    return nc


def get_nc():
    key = ("v2", WARM)
    if key not in _NC_CACHE:
        _NC_CACHE[key] = build_nc()
    return _NC_CACHE[key]


def make_in_maps(Q, V, freqs):
    Q = np.asarray(Q, dtype=np.float32).reshape(B * H, T, NDIM).astype(np.float16)
    V = np.asarray(V, dtype=np.float32).reshape(B * H, T, DV).astype(np.float16)
    cosv, ssw = _host_tables(freqs)
    in_maps = []
    for c in range(NCORES):
        in_maps.append(
            {
                "q": np.ascontiguousarray(Q[BH_PER_CORE * c : BH_PER_CORE * (c + 1)]),
                "v": np.ascontiguousarray(V[BH_PER_CORE * c : BH_PER_CORE * (c + 1)]),
                "cosv": cosv,
                "ssw": ssw,
            }
        )
    return in_maps


def kernel(Q, V, freqs):
    global LAST_RESULTS
    from concourse.bass_utils import run_bass_kernel_spmd

    nc = get_nc()
    in_maps = make_in_maps(Q, V, freqs)
    res = run_bass_kernel_spmd(
        nc, in_maps, core_ids=list(range(NCORES)), trace=TRACE
    )
    LAST_RESULTS = res
    out = np.stack([r["o"] for r in res.results])  # [8, 2, T, DV]
    return out.reshape(B, H, T, DV).astype(np.float32)
